# revision 29
# baseline (speedup 1.0000x reference)
"""Trainium2 Bass kernel for nn_GATRegressor (2-layer edge-featured GAT + pool + MLP).

Strategy (8-core SPMD, single NEFF):
  - Nodes partitioned into 8 graph-aligned ranges; edges partitioned by dst.
  - Per 128-dst-node tile: indirect-DMA gather of [h | al_src] rows by edge src,
    attention alpha computed on-chip (leaky-relu/exp without segment-max --
    mathematically identical softmax), scatter-add via one-hot selection-matrix
    matmul on the PE accumulating in PSUM (denominators ride along as extra
    columns). Per-head transform W applied AFTER aggregation (linearity), with
    0.25/denom folded in before the transform.
  - One AllGather of the per-core [h|al_s|al_d] shard between layers; one tiny
    AllReduce for the mean-edge-attr self-loop term.
  - Global mean-pool + 2-layer MLP on-device (counts division after W_r1 matmul).

Host side does only index/layout preprocessing (sort edges by dst, remap ids,
pad, interleave) and small weight repacking; all FLOP-heavy math runs on device.
"""
import os
import numpy as np
from contextlib import ExitStack

BODY_LEVEL = int(os.environ.get('KBODY', '9'))

import concourse.bass as bass
import concourse.bacc as bacc
import concourse.mybir as mybir
import concourse.tile as tile
from concourse.bass import IndirectOffsetOnAxis
from concourse.bass_utils import run_bass_kernel_spmd

F32 = mybir.dt.float32
I32 = mybir.dt.int32
AF = mybir.ActivationFunctionType
OP = mybir.AluOpType
P = 128
NEG = 0.2


def make_cfg(N=100000, E=400000, G=4096, NC=8, TPC=98, GPC=640, L=2, IN=64, HID=64, ED=12, H=4):
    cfg = dict(N=N, E=E, G=G, NC=NC, TPC=TPC, GPC=GPC, L=L, IN=IN, HID=HID, ED=ED, H=H)
    cfg['NPC'] = TPC * P
    cfg['NT'] = NC * cfg['NPC']
    cfg['SENT'] = cfg['NT']
    cfg['NWIN'] = GPC // P
    return cfg


FULL_CFG = make_cfg()


def _finish(nc):
    nc.finalize()
    return nc


# --------------------------------------------------------------------------
# host preprocessing: index/layout only (+ tiny weight repacking)
# --------------------------------------------------------------------------
def preprocess(inputs, cfg):
    N, E, G, NC, TPC, GPC = cfg['N'], cfg['E'], cfg['G'], cfg['NC'], cfg['TPC'], cfg['GPC']
    NPC, NT, SENT, L, H, ED = cfg['NPC'], cfg['NT'], cfg['SENT'], cfg['L'], cfg['H'], cfg['ED']
    HID = cfg['HID']

    x = np.ascontiguousarray(np.asarray(inputs['x'], np.float32))
    ei = np.asarray(inputs['edge_index'])
    ea = np.ascontiguousarray(np.asarray(inputs['edge_attr'], np.float32))
    batch = np.asarray(inputs['batch'])

    W_in = np.asarray(inputs['W_in'], np.float32)
    b_in = np.asarray(inputs['b_in'], np.float32)
    W_gat = np.asarray(inputs['W_gat'], np.float32)
    att_src = np.asarray(inputs['att_src'], np.float32)
    att_dst = np.asarray(inputs['att_dst'], np.float32)
    W_edge = np.asarray(inputs['W_edge'], np.float32)
    att_edge = np.asarray(inputs['att_edge'], np.float32)
    b_gat = np.asarray(inputs['b_gat'], np.float32)
    W_r1 = np.asarray(inputs['W_r1'], np.float32)
    b_r1 = np.asarray(inputs['b_r1'], np.float32)
    W_r2 = np.asarray(inputs['W_r2'], np.float32)
    b_r2 = np.asarray(inputs['b_r2'], np.float32)

    # ---- weight packing ----
    # W_ext_l [64, 264] = [0.25*W (256) | As_l (4) | Ad_l (4)]
    W_ext = np.zeros((L, HID, 4 * HID + 2 * H), np.float32)
    for l in range(L):
        Wl = W_gat[l].reshape(HID, H, HID)
        W_ext[l, :, :4 * HID] = 0.25 * W_gat[l]
        for h in range(H):
            W_ext[l, :, 4 * HID + h] = np.einsum('kc,c->k', Wl[:, h, :], att_src[l, h])
            W_ext[l, :, 4 * HID + H + h] = np.einsum('kc,c->k', Wl[:, h, :], att_dst[l, h])
    Ae = np.zeros((ED, L * H), np.float32)
    for l in range(L):
        Wl = W_edge[l].reshape(ED, H, HID)
        for h in range(H):
            Ae[:, l * H + h] = np.einsum('dc,c->d', Wl[:, h, :], att_edge[l, h])
    W_in_ext = np.ascontiguousarray(np.concatenate([W_in, b_in[None, :]], axis=0))

    # ---- graph-aligned node shard cuts ----
    gcounts = np.bincount(batch, minlength=G)
    cum = np.cumsum(gcounts)
    cuts_g = [0]
    for c in range(1, NC):
        cuts_g.append(int(np.searchsorted(cum, c * N / NC)))
    cuts_g.append(G)
    cuts_g = np.array(cuts_g)
    assert np.all(np.diff(cuts_g) > 0) and np.all(np.diff(cuts_g) <= GPC), np.diff(cuts_g)
    cuts_n = np.array([0] + [int(cum[cuts_g[c] - 1]) for c in range(1, NC)] + [N])
    npc_real = np.diff(cuts_n)
    assert np.all(npc_real <= NPC), npc_real

    core_of = (np.searchsorted(cuts_n, np.arange(N), side='right') - 1).astype(np.int64)
    slot_of = np.arange(N) - cuts_n[core_of]
    pad_id = (core_of * NPC + slot_of).astype(np.int32)

    # ---- edge sorting by (dst core, dst slot) ----
    src, dst = ei[0].astype(np.int64), ei[1].astype(np.int64)
    dcore = core_of[dst]
    dslot = slot_of[dst]
    order = np.lexsort((dslot, dcore))
    s_src = src[order]
    s_dslot = dslot[order]
    s_dcore = dcore[order]
    s_ea = ea[order]

    tile_of = s_dcore * TPC + (s_dslot // P)
    tcnt = np.bincount(tile_of, minlength=NC * TPC).reshape(NC, TPC)
    ct = np.maximum(np.ceil(tcnt / P).astype(np.int64).max(axis=0), 1)   # [TPC]
    CH = int(ct.sum())
    SLOT = CH * P
    ch_off = np.concatenate([[0], np.cumsum(ct)]).astype(np.int64)

    src_idx = np.full((NC, SLOT), SENT, np.int32)
    dstf = np.full((NC, SLOT), 999.0, np.float32)
    ea_slot = np.zeros((NC, SLOT, ED), np.float32)

    tile_starts = np.concatenate([[0], np.cumsum(tcnt.reshape(-1))])
    for c in range(NC):
        for t in range(TPC):
            gt = c * TPC + t
            e0, e1 = tile_starts[gt], tile_starts[gt + 1]
            n = e1 - e0
            s0 = ch_off[t] * P
            if n:
                src_idx[c, s0:s0 + n] = pad_id[s_src[e0:e1]]
                dstf[c, s0:s0 + n] = (s_dslot[e0:e1] % P).astype(np.float32)
                ea_slot[c, s0:s0 + n] = s_ea[e0:e1]

    def interleave(a):  # [NC, SLOT, ...] -> [NC, P, CH, ...] with slot s=(j*P+p) -> [p, j]
        return np.ascontiguousarray(
            a.reshape(NC, CH, P, *a.shape[2:]).transpose(0, 2, 1, *range(3, a.ndim + 1)))

    src_idx_il = interleave(src_idx)
    dstf_il = interleave(dstf)
    dstf_rows = dstf.reshape(NC, 1, SLOT)  # slot-major: [1, j*128+p]
    ea_sT = np.ascontiguousarray(ea_slot.transpose(0, 2, 1))  # [NC, 12, SLOT]

    # x transposed + ones row, per-core slices [NC, 65, NPC]
    xT = np.zeros((NC, cfg['IN'] + 1, NPC), np.float32)
    xT[:, cfg['IN']] = 1.0
    xTg = xT.reshape(NC * NPC * 0 + NC, cfg['IN'] + 1, NPC)  # alias
    for c in range(NC):
        sl = slice(cuts_n[c], cuts_n[c + 1])
        xTg[c, :cfg['IN'], :npc_real[c]] = x[sl].T

    batch_local = np.full((NC, NPC, 1), -1.0, np.float32)
    for c in range(NC):
        sl = slice(cuts_n[c], cuts_n[c + 1])
        batch_local[c, :npc_real[c], 0] = (batch[sl] - cuts_g[c]).astype(np.float32)

    wr1e = np.zeros((HID + 1, HID + 1), np.float32)
    wr1e[:HID, :HID] = W_r1
    wr1e[HID, HID] = 1.0
    return dict(W_in_ext=W_in_ext, W_ext=W_ext, Ae=Ae, b_gat=b_gat,
                W_r1=np.ascontiguousarray(wr1e), b_r1=b_r1,
                W_r2row=np.ascontiguousarray(W_r2[:, 0][None, :]), b_r2=float(b_r2.reshape(-1)[0]),
                xT=xT, src_idx=src_idx_il, dstf=dstf_il, dstf_rows=dstf_rows,
                ea_sT=ea_sT, batch_local=batch_local,
                ct=[int(v) for v in ct], ch_off=[int(v) for v in ch_off], CH=CH,
                cuts_g=cuts_g, npc_real=npc_real)


# --------------------------------------------------------------------------
# bass program
# --------------------------------------------------------------------------
def build(cfg, ct, ch_off, CH, b_r2, stage='full'):
    N, E, NC, TPC, GPC, L, H, ED = (cfg['N'], cfg['E'], cfg['NC'], cfg['TPC'],
                                    cfg['GPC'], cfg['L'], cfg['H'], cfg['ED'])
    NPC, NT, NWIN, IN, HID = cfg['NPC'], cfg['NT'], cfg['NWIN'], cfg['IN'], cfg['HID']
    HP = H * HID            # 256
    WE_C = 4 * HID + 2 * H  # 264
    RC = H * (HID + 1)      # 260 rhs/psum_B columns

    nc = bacc.Bacc(None, num_devices=NC)

    # ---- I/O ----
    t_xT = nc.dram_tensor('xt', [IN + 1, NPC], F32, kind='ExternalInput')
    t_winext = nc.dram_tensor('winext', [IN + 1, HID], F32, kind='ExternalInput')
    t_wext = nc.dram_tensor('wext', [L, HID, WE_C], F32, kind='ExternalInput')
    t_ae = nc.dram_tensor('ae', [ED, L * H], F32, kind='ExternalInput')
    t_bgat = nc.dram_tensor('bgat', [L, HID], F32, kind='ExternalInput')
    t_wr1 = nc.dram_tensor('wr1', [HID + 1, HID + 1], F32, kind='ExternalInput')
    t_br1 = nc.dram_tensor('br1', [1, HID], F32, kind='ExternalInput')
    t_wr2 = nc.dram_tensor('wr2', [1, HID], F32, kind='ExternalInput')
    t_srci = nc.dram_tensor('srci', [P, CH], I32, kind='ExternalInput')
    t_dstf = nc.dram_tensor('dstf', [P, CH], F32, kind='ExternalInput')
    t_dstfr = nc.dram_tensor('dstfr', [1, CH * P], F32, kind='ExternalInput')
    t_east = nc.dram_tensor('east', [ED, CH * P], F32, kind='ExternalInput')
    t_batch = nc.dram_tensor('batchl', [NPC, 1], F32, kind='ExternalInput')
    t_out = nc.dram_tensor('out', [GPC, 1], F32, kind='ExternalOutput')

    # ---- internal DRAM ----
    hx_sh = [nc.dram_tensor(f'hx_sh{l}', [NPC, 72], F32) for l in range(L)]
    hx_full = [nc.dram_tensor(f'hx_full{l}', [NT + 1, 72], F32)
               for l in range(L)]
    hx_last = nc.dram_tensor('hx_last', [NPC, HID + 1], F32)
    t_ale = nc.dram_tensor('ale', [P, L, CH, H], F32)
    ar_in = nc.dram_tensor('ar_in', [1, L * H], F32)
    ar_out = nc.dram_tensor('ar_out', [1, L * H], F32)

    groups = [list(range(NC))]

    with tile.TileContext(nc) as tc, ExitStack() as ctx:
        cpool = ctx.enter_context(tc.tile_pool(name='const', bufs=1))

        # ---------- constants ----------
        iotaG_const = nc.inline_tensor(
            np.broadcast_to(np.arange(GPC, dtype=np.float32), (P, GPC)).copy(), name='iotaG_c')
        iotac_const = nc.inline_tensor(np.arange(P, dtype=np.float32)[:, None].copy(),
                                       name='iotac_c')
        iotaG = cpool.tile([P, GPC], F32)
        nc.sync.dma_start(out=iotaG[:], in_=iotaG_const[:, :])
        iota128 = iotaG[:, 0:P]
        iota_col = cpool.tile([P, 1], F32)
        nc.sync.dma_start(out=iota_col[:], in_=iotac_const[:, :])
        ident = cpool.tile([P, P], F32)
        nc.vector.tensor_scalar(out=ident[:], in0=iota128, scalar1=iota_col[:, :1],
                                scalar2=None, op0=OP.is_equal)
        ones_col = cpool.tile([P, 1], F32)
        nc.vector.memset(ones_col[:], 1.0)

        w_in_sb = cpool.tile([IN + 1, HID], F32)
        nc.sync.dma_start(out=w_in_sb[:], in_=t_winext[:, :])
        wext_sb = []
        for l in range(L):
            # W_ext duplicated into partitions [0:64] and [64:128] so per-head
            # matmul lhsT slices (base partition 0 or 64) find a matching rhs.
            wl = cpool.tile([2 * HID, WE_C], F32, tag=f'wext{l}', name=f'wext_sb{l}')
            nc.sync.dma_start(out=wl[0:HID, :], in_=t_wext[l, :, :])
            nc.sync.dma_start(out=wl[HID:2 * HID, :], in_=t_wext[l, :, :])
            wext_sb.append(wl)
        ae_sb = cpool.tile([ED, L * H], F32)
        nc.sync.dma_start(out=ae_sb[:], in_=t_ae[:, :])
        wr1_sb = cpool.tile([HID + 1, HID + 1], F32)
        nc.sync.dma_start(out=wr1_sb[:], in_=t_wr1[:, :])

        ones_row = cpool.tile([1, P], F32)
        nc.vector.memset(ones_row[:], 1.0)

        def bcast_row(dst_tile, src_row_ap, width, pool):
            """dst[p, :] = src_row for all p, via K=1 matmul (ones ⊗ row)."""
            pbc = pool.tile([P, max(width, 256)], F32, tag='pT', name='pbc', space='PSUM')
            nc.tensor.matmul(out=pbc[:, 0:width], lhsT=ones_row[:], rhs=src_row_ap,
                             start=True, stop=True)
            nc.vector.tensor_copy(out=dst_tile[:], in_=pbc[:, 0:width])

        bgat_row = cpool.tile([1, L * HID], F32)
        nc.sync.dma_start(out=bgat_row[:], in_=t_bgat[:, :].rearrange('l c -> (l c)')[None, :])
        bgat_bc = cpool.tile([P, L * HID], F32)
        br1_row = cpool.tile([1, HID], F32)
        nc.sync.dma_start(out=br1_row[:], in_=t_br1[:, :])
        br1_bc = cpool.tile([P, HID], F32)
        wr2_row = cpool.tile([1, HID], F32)
        nc.sync.dma_start(out=wr2_row[:], in_=t_wr2[:, :])
        wr2_bc = cpool.tile([P, HID], F32)

        # sentinel row of hx_full: al_s=-1e9 kills padded edges (alpha=0), and the
        # h part only ever gets multiplied by that alpha=0, so -1e9 everywhere is fine.
        sent_row = cpool.tile([1, 72], F32)
        nc.vector.memset(sent_row[:], -1.0e9)
        for l in range(L):
            nc.gpsimd.dma_start(out=hx_full[l][NT:NT + 1, :], in_=sent_row[:])

        with tc.tile_pool(name='psA', bufs=2, space='PSUM') as psA, \
             tc.tile_pool(name='psB', bufs=1, space='PSUM') as psB, \
             tc.tile_pool(name='sb', bufs=6) as sb, \
             tc.tile_pool(name='gat', bufs=12) as gat:

            bcast_row(bgat_bc, bgat_row[:], L * HID, psA)
            bcast_row(br1_bc, br1_row[:], HID, psA)
            bcast_row(wr2_bc, wr2_row[:], HID, psA)

            # ---------- phase ALE: al_e for local edge slots (both layers) + local sum ----------
            ale_acc = cpool.tile([P, L * H], F32)
            nc.vector.memset(ale_acc[:], 0.0)
            GRP = 8
            for g0 in range(0, CH, GRP):
                gg = min(GRP, CH - g0)
                ea8 = sb.tile([ED, GRP * P], F32, tag='ea8')
                nc.sync.dma_start(out=ea8[:, 0:gg * P], in_=t_east[:, g0 * P:(g0 + gg) * P])
                acc8 = sb.tile([P, GRP, L * H], F32, tag='acc8')
                for jj in range(gg):
                    pale = psB.tile([P, 72], F32, tag='pO', space='PSUM')
                    nc.tensor.matmul(out=pale[:, 0:L * H], lhsT=ea8[:, jj * P:(jj + 1) * P],
                                     rhs=ae_sb[:], start=True, stop=True)
                    nc.vector.tensor_copy(out=acc8[:, jj, :], in_=pale[:, 0:L * H])
                # write both layers' slices
                for l in range(L):
                    nc.sync.dma_start(out=t_ale[:, l, g0:g0 + gg, :],
                                      in_=acc8[:, 0:gg, l * H:(l + 1) * H])
                # accumulate local sum: reduce over jj
                tmp8 = sb.tile([P, L * H], F32, tag='tmp8')
                nc.vector.tensor_reduce(out=tmp8[:], in_=acc8[:, 0:gg, :].rearrange('p j h -> p h j'),
                                        op=OP.add, axis=mybir.AxisListType.X)
                nc.vector.tensor_add(out=ale_acc[:], in0=ale_acc[:], in1=tmp8[:])
            # cross-partition sum -> [1, 8] -> AllReduce -> mean -> broadcast
            pm = psB.tile([P, 72], F32, tag='pO', space='PSUM')
            nc.tensor.matmul(out=pm[0:1, 0:L * H], lhsT=ones_col[:], rhs=ale_acc[:],
                             start=True, stop=True)
            m_sb = cpool.tile([1, L * H], F32)
            nc.vector.tensor_copy(out=m_sb[:], in_=pm[0:1, 0:L * H])
            nc.sync.dma_start(out=ar_in[:, :], in_=m_sb[:])
            tc.strict_bb_all_engine_barrier()
            nc.gpsimd.collective_compute('AllReduce', OP.add, replica_groups=groups,
                                         ins=[ar_in[:, :]], outs=[ar_out[:, :]])
            tc.strict_bb_all_engine_barrier()
            mr_sb = cpool.tile([1, L * H], F32)
            nc.sync.dma_start(out=mr_sb[:], in_=ar_out[:, :])
            nc.scalar.mul(mr_sb[:], mr_sb[:], 1.0 / E)
            alem_bc = cpool.tile([P, L * H], F32)
            bcast_row(alem_bc, mr_sb[:], L * H, psA)

            if stage == 'ale':
                nc.sync.dma_start(out=t_out[0:1, :], in_=m_sb[:, 0:1])

            # ---------- phase 0: h0 = elu(x @ W_in + b_in), al_s/al_d ----------
            for t in (range(TPC) if stage != 'ale' else ()):
                rows = slice(t * P, (t + 1) * P)
                xt = sb.tile([IN + 1, P], F32, tag='xt')
                nc.sync.dma_start(out=xt[:], in_=t_xT[:, rows])
                pb = psA.tile([P, RC], F32, tag='pB', space='PSUM')
                nc.tensor.matmul(out=pb[:, 0:HID], lhsT=xt[:], rhs=w_in_sb[:],
                                 start=True, stop=True)
                hn, hnT, alsd = _elu_transform(nc, sb, psA, psB, pb[:, 0:HID], None,
                                              ident, wext_sb[0], HID)
                nc.sync.dma_start(out=hx_sh[0][rows, 0:64], in_=hn[:])
                nc.sync.dma_start(out=hx_sh[0][rows, 64:72], in_=alsd[:])

            # AllGather layer-0 table
            if stage == 'ale':
                pass
            tc.strict_bb_all_engine_barrier()
            nc.gpsimd.collective_compute('AllGather', OP.bypass, replica_groups=groups,
                                         ins=[hx_sh[0][:, :]], outs=[hx_full[0][0:NT, :]])
            tc.strict_bb_all_engine_barrier()

            if stage == 'phase0':
                probe = sb.tile([1, 72], F32, tag='probe', name='probe')
                nc.sync.dma_start(out=probe[:], in_=hx_full[0][123:124, :])
                nc.sync.dma_start(out=t_out[0:1, :], in_=probe[:, 0:1])

            # ---------- GAT layers ----------
            ntile_cap = TPC
            lstage = stage
            if stage.startswith('layer0t'):
                ntile_cap = int(stage[len('layer0t'):])
                lstage = 'layer0'
            for l in (range(L) if lstage in ('layer0', 'full') else ()):
                if lstage == 'layer0' and l == 1:
                    probe = sb.tile([1, 72], F32, tag='probe', name='probe')
                    nc.sync.dma_start(out=probe[:], in_=hx_full[1][123:124, :])
                    nc.sync.dma_start(out=t_out[0:1, :], in_=probe[:, 0:1])
                    break
                last = (l == L - 1)
                for t in range(min(TPC, ntile_cap) if lstage == 'layer0' else TPC):
                    rows = slice(t * P, (t + 1) * P)
                    ctt = ct[t]
                    j0 = ch_off[t]
                    selft = sb.tile([P, 72], F32, tag='selft')
                    nc.sync.dma_start(out=selft[:], in_=hx_sh[l][rows, :])
                    srct = sb.tile([P, ctt], I32, tag='srct')
                    nc.sync.dma_start(out=srct[:], in_=t_srci[:, j0:j0 + ctt])
                    dft = sb.tile([P, ctt], F32, tag='dft')
                    nc.sync.dma_start(out=dft[:], in_=t_dstf[:, j0:j0 + ctt])
                    dfr = sb.tile([1, ctt * P], F32, tag='dfr')
                    nc.sync.dma_start(out=dfr[:], in_=t_dstfr[:, j0 * P:(j0 + ctt) * P])
                    alet = sb.tile([P, ctt, H], F32, tag='alet')
                    nc.sync.dma_start(out=alet[:], in_=t_ale[:, l, j0:j0 + ctt, :])

                    # gather [h|al_s|al_d] rows by edge src (one chunk per call:
                    # HW indirect DMA = one row index per partition)
                    Gt = gat.tile([P, ctt, 72], F32, tag='G')
                    for jj in range(ctt):
                        nc.gpsimd.indirect_dma_start(
                            out=Gt[:, jj, :], out_offset=None, in_=hx_full[l][:, :],
                            in_offset=IndirectOffsetOnAxis(ap=srct[:, jj:jj + 1], axis=0))

                    if BODY_LEVEL < 2:
                        continue
                    # al_d per edge: expand this tile's al_d rows through the
                    # transposed one-hot (dst_local) matrix on the PE
                    pD = psB.tile([P, 32], F32, tag='pD', space='PSUM')
                    for jj in range(ctt):
                        pDst = psA.tile([P, HP], F32, tag='pT', name='pDst', space='PSUM')
                        nc.tensor.matmul(out=pDst[:, 0:P], lhsT=ones_row[:],
                                         rhs=dfr[:, jj * P:(jj + 1) * P],
                                         start=True, stop=True)
                        ST = sb.tile([P, P], F32, tag='ST')
                        nc.vector.tensor_scalar(out=ST[:], in0=pDst[:, 0:P],
                                                scalar1=iota_col[:, :1], scalar2=None,
                                                op0=OP.is_equal)
                        nc.tensor.matmul(out=pD[:, 4 * jj:4 * jj + 4], lhsT=ST[:],
                                         rhs=selft[:, 68:72], start=True, stop=True)

                    if BODY_LEVEL < 3:
                        continue
                    # alpha-hat per edge
                    lg = sb.tile([P, ctt, H], F32, tag='lg')
                    nc.vector.tensor_add(out=lg[:], in0=Gt[:, :, 64:68],
                                         in1=pD[:, 0:4 * ctt].rearrange('p (j h) -> p j h', h=H))
                    nc.vector.tensor_add(out=lg[:], in0=lg[:], in1=alet[:])
                    nc.vector.scalar_tensor_tensor(out=lg[:], in0=lg[:], scalar=NEG,
                                                   in1=lg[:], op0=OP.mult, op1=OP.max)
                    ah = sb.tile([P, ctt, H], F32, tag='ah')
                    nc.scalar.activation(ah[:], lg[:], AF.Exp)

                    # rhs = per-head [alpha*h (64) | alpha]
                    rhs = sb.tile([P, ctt, RC], F32, tag='rhs')
                    rhs_r = rhs[:].rearrange('p j (h c) -> p j h c', h=H)
                    for h in range(H):
                        nc.vector.tensor_mul(
                            out=rhs_r[:, :, h, 0:HID], in0=Gt[:, :, 0:HID],
                            in1=ah[:, :, h:h + 1].to_broadcast([P, ctt, HID]))
                    nc.vector.tensor_copy(out=rhs_r[:, :, :, HID], in_=ah[:])

                    if BODY_LEVEL < 4:
                        continue
                    pb = psA.tile([P, RC], F32, tag='pB', space='PSUM')
                    for jj in range(ctt):
                        S = sb.tile([P, P], F32, tag='S')
                        nc.vector.tensor_scalar(out=S[:], in0=iota128,
                                                scalar1=dft[:, jj:jj + 1], scalar2=None,
                                                op0=OP.is_equal)
                        nc.tensor.matmul(out=pb[:], lhsT=S[:], rhs=rhs[:, jj, :],
                                         start=(jj == 0), stop=(jj == ctt - 1))

                    if BODY_LEVEL < 5:
                        continue
                    # self-loop alpha
                    sl = sb.tile([P, H], F32, tag='sl')
                    nc.vector.tensor_add(out=sl[:], in0=selft[:, 64:68], in1=selft[:, 68:72])
                    nc.vector.tensor_add(out=sl[:], in0=sl[:],
                                         in1=alem_bc[:, l * H:(l + 1) * H])
                    nc.vector.scalar_tensor_tensor(out=sl[:], in0=sl[:], scalar=NEG,
                                                   in1=sl[:], op0=OP.mult, op1=OP.max)
                    ahs = sb.tile([P, H], F32, tag='ahs')
                    nc.scalar.activation(ahs[:], sl[:], AF.Exp)

                    pb_r = pb[:].rearrange('p (h c) -> p h c', h=H)
                    dn = sb.tile([P, H], F32, tag='dn')
                    nc.vector.tensor_add(out=dn[:], in0=pb_r[:, :, HID], in1=ahs[:])
                    nc.vector.tensor_scalar_add(dn[:], dn[:], 1.0e-16)
                    rc = sb.tile([P, H], F32, tag='rc')
                    nc.vector.reciprocal(rc[:], dn[:])

                    # Bw = (B + h_self * ahs) * recip   [128, 4, 64]
                    bw = sb.tile([P, H, HID], F32, tag='bw')
                    nc.vector.tensor_mul(
                        out=bw[:],
                        in0=selft[:, 0:64].rearrange('p (o c) -> p o c', o=1).to_broadcast([P, H, HID]),
                        in1=ahs[:].rearrange('p (h o) -> p h o', h=H).to_broadcast([P, H, HID]))
                    nc.vector.tensor_add(out=bw[:], in0=bw[:], in1=pb_r[:, :, 0:HID])
                    nc.vector.tensor_mul(
                        out=bw[:], in0=bw[:],
                        in1=rc[:].rearrange('p (h o) -> p h o', h=H).to_broadcast([P, H, HID]))

                    if BODY_LEVEL < 6:
                        continue
                    # transpose Bw head-by-head ([128,64] -> [64,128], full psum
                    # tiles: transpose-mode into column-sliced PSUM fails on HW),
                    # then per-head transform accumulating into one psum
                    bt = sb.tile([HID, H * P], F32, tag='bt')
                    for h in range(H):
                        pTh = psA.tile([HID, P], F32, tag='pN', name='pTh', space='PSUM')
                        nc.tensor.transpose(out=pTh[:], in_=bw[:, h, :], identity=ident[:])
                        nc.vector.tensor_copy(out=bt[:, h * P:(h + 1) * P], in_=pTh[:])
                    pO = psB.tile([P, 72], F32, tag='pO', space='PSUM')
                    for h in range(H):
                        nc.tensor.matmul(out=pO[:, 0:HID],
                                         lhsT=bt[:, h * P:(h + 1) * P],
                                         rhs=wext_sb[l][0:HID, h * HID:(h + 1) * HID],
                                         start=(h == 0), stop=(h == H - 1))

                    if BODY_LEVEL < 7:
                        continue
                    if not last:
                        hn, hnT, alsd = _elu_transform(nc, sb, psA, psB, pO[:, 0:HID],
                                                       bgat_bc[:, l * HID:(l + 1) * HID],
                                                       ident, wext_sb[l + 1], HID)
                        nc.sync.dma_start(out=hx_sh[l + 1][rows, 0:64], in_=hn[:])
                        nc.sync.dma_start(out=hx_sh[l + 1][rows, 64:72], in_=alsd[:])
                    else:
                        hn = _elu_only(nc, sb, pO[:, 0:HID], bgat_bc[:, l * HID:(l + 1) * HID])
                        hp = sb.tile([P, HID + 1], F32, tag='hp')
                        nc.vector.tensor_copy(out=hp[:, 0:HID], in_=hn[:])
                        nc.vector.memset(hp[:, HID:HID + 1], 1.0)
                        nc.sync.dma_start(out=hx_last[rows, :], in_=hp[:])

                if not last:
                    tc.strict_bb_all_engine_barrier()
                    nc.gpsimd.collective_compute(
                        'AllGather', OP.bypass, replica_groups=groups,
                        ins=[hx_sh[l + 1][:, :]], outs=[hx_full[l + 1][0:NT, :]])
                    tc.strict_bb_all_engine_barrier()

        # ---------- pooling + MLP ----------
        if stage == 'full':
            _pooling_phase(nc, tc, cfg, t_out, t_batch, hx_last, iotaG, ident,
                           wr1_sb, br1_bc, wr2_bc, b_r2)
    nc.finalize()
    return nc


def _pooling_phase(nc, tc, cfg, t_out, t_batch, hx_last, iotaG, ident,
                   wr1_sb, br1_bc, wr2_bc, b_r2):
        NC, TPC, GPC, HID = cfg['NC'], cfg['TPC'], cfg['GPC'], cfg['HID']
        NWIN = cfg['NWIN']
        with tc.tile_pool(name='psP', bufs=1, space='PSUM') as psP, \
             tc.tile_pool(name='sbp', bufs=3) as sbp:
            NWA = min(4, NWIN)  # graphs columns in first psum bank group (512)
            ppa = psP.tile([HID + 1, NWA * P], F32, tag='ppa', space='PSUM')
            if NWIN > NWA:
                ppb = psP.tile([HID + 1, (NWIN - NWA) * P], F32, tag='ppb',
                               name='ppb', space='PSUM')
            else:
                ppb = None
            for t in range(TPC):
                rows = slice(t * P, (t + 1) * P)
                lh = sbp.tile([P, HID + 1], F32, tag='lh')
                nc.sync.dma_start(out=lh[:], in_=hx_last[rows, :])
                bt_ = sbp.tile([P, 1], F32, tag='btl')
                nc.sync.dma_start(out=bt_[:], in_=t_batch[rows, :])
                Sb = sbp.tile([P, GPC], F32, tag='Sb')
                nc.vector.tensor_scalar(out=Sb[:], in0=iotaG[:], scalar1=bt_[:, :1],
                                        scalar2=None, op0=OP.is_equal)
                nc.tensor.matmul(out=ppa[:], lhsT=lh[:], rhs=Sb[:, 0:NWA * P],
                                 start=(t == 0), stop=(t == TPC - 1))
                if ppb is not None:
                    nc.tensor.matmul(out=ppb[:], lhsT=lh[:], rhs=Sb[:, NWA * P:],
                                     start=(t == 0), stop=(t == TPC - 1))
            pool_sb = sbp.tile([HID + 1, GPC], F32, tag='pool')
            nc.vector.tensor_copy(out=pool_sb[:, 0:NWA * P], in_=ppa[:])
            if ppb is not None:
                nc.vector.tensor_copy(out=pool_sb[:, NWA * P:], in_=ppb[:])

            for w in range(NWIN):
                wcols = slice(w * P, (w + 1) * P)
                # [128g, 65] = pool[0:65, wcols].T @ [W_r1 | e64]: graphs land on
                # partitions directly (no transpose op), col 64 = counts
                pt = psP.tile([P, HID + 1], F32, tag='pt', space='PSUM')
                nc.tensor.matmul(out=pt[:], lhsT=pool_sb[0:HID + 1, wcols],
                                 rhs=wr1_sb[:], start=True, stop=True)
                cnt = sbp.tile([P, 1], F32, tag='cnt')
                nc.vector.tensor_scalar_max(cnt[:], pt[:, HID:HID + 1], 1.0)
                rcc = sbp.tile([P, 1], F32, tag='rcc')
                nc.vector.reciprocal(rcc[:], cnt[:])
                rp = sbp.tile([P, HID], F32, tag='rp')
                nc.vector.tensor_scalar(out=rp[:], in0=pt[:, 0:HID], scalar1=rcc[:, :1],
                                        scalar2=None, op0=OP.mult)
                nc.vector.tensor_add(out=rp[:], in0=rp[:], in1=br1_bc[:])
                rr = sbp.tile([P, HID], F32, tag='rr')
                nc.scalar.activation(rr[:], rp[:], AF.Relu)
                scr = sbp.tile([P, HID], F32, tag='scr')
                nc.vector.tensor_mul(out=scr[:], in0=rr[:], in1=wr2_bc[:])
                ov = sbp.tile([P, 1], F32, tag='ov')
                nc.vector.tensor_reduce(out=ov[:], in_=scr[:], op=OP.add,
                                        axis=mybir.AxisListType.X)
                nc.vector.tensor_scalar_add(ov[:], ov[:], b_r2)
                nc.sync.dma_start(out=t_out[w * P:(w + 1) * P, :], in_=ov[:])


def _elu_transform(nc, sb, psA, psB, z_psum, bias_bc, ident, wext_next, HID):
    """elu(z_psum + bias) -> hn; transpose -> hnT; alsd = hnT.T @ wext_next[:,256:264]."""
    z = sb.tile([P, HID], F32, tag='z')
    if bias_bc is None:
        nc.vector.tensor_copy(out=z[:], in_=z_psum)
    else:
        nc.vector.tensor_add(out=z[:], in0=z_psum, in1=bias_bc)
    hn = _elu_only(nc, sb, z[:], None)
    pN = psA.tile([HID, P], F32, tag='pN', space='PSUM')
    nc.tensor.transpose(out=pN[:], in_=hn[:], identity=ident[:])
    hnT = sb.tile([HID, P], F32, tag='hnT')
    nc.vector.tensor_copy(out=hnT[:], in_=pN[:])
    pO = psB.tile([P, 72], F32, tag='pO', space='PSUM')
    nc.tensor.matmul(out=pO[:, 64:72], lhsT=hnT[:], rhs=wext_next[0:HID, 4 * HID:4 * HID + 8],
                     start=True, stop=True)
    alsd = sb.tile([P, 8], F32, tag='alsd')
    nc.vector.tensor_copy(out=alsd[:], in_=pO[:, 64:72])
    return hn, hnT, alsd


def _elu_only(nc, sb, z_ap, bias_bc):
    """elu(z + bias): relu(z)-1 + exp(min(z,0))."""
    HIDW = z_ap.shape[-1]
    if bias_bc is not None:
        z = sb.tile([P, HIDW], F32, tag='z')
        nc.vector.tensor_add(out=z[:], in0=z_ap, in1=bias_bc)
        z_ap = z[:]
    tm = sb.tile([P, HIDW], F32, tag='tm')
    nc.vector.tensor_scalar_min(tm[:], z_ap, 0.0)
    ex = sb.tile([P, HIDW], F32, tag='ex')
    nc.scalar.activation(ex[:], tm[:], AF.Exp)
    hn = sb.tile([P, HIDW], F32, tag='hn')
    nc.vector.tensor_scalar(out=hn[:], in0=z_ap, scalar1=0.0, scalar2=-1.0,
                            op0=OP.max, op1=OP.add)
    nc.vector.tensor_add(out=hn[:], in0=hn[:], in1=ex[:])
    return hn


# --------------------------------------------------------------------------
# entry point
# --------------------------------------------------------------------------
def _in_maps(pp, cfg):
    NC = cfg['NC']
    shared = dict(winext=pp['W_in_ext'], wext=pp['W_ext'], ae=pp['Ae'],
                  bgat=pp['b_gat'], wr1=pp['W_r1'], br1=pp['b_r1'][None, :],
                  wr2=pp['W_r2row'])
    maps = []
    for c in range(NC):
        m = dict(shared)
        m['xt'] = pp['xT'][c]
        m['srci'] = pp['src_idx'][c]
        m['dstf'] = pp['dstf'][c]
        m['dstfr'] = pp['dstf_rows'][c]
        m['east'] = pp['ea_sT'][c]
        m['batchl'] = pp['batch_local'][c]
        maps.append({k: np.ascontiguousarray(v) for k, v in m.items()})
    return maps


def run(inputs, cfg=None, trace=False):
    cfg = cfg or FULL_CFG
    pp = preprocess(inputs, cfg)
    nc = build(cfg, pp['ct'], pp['ch_off'], pp['CH'], pp['b_r2'])
    res = run_bass_kernel_spmd(nc, _in_maps(pp, cfg), core_ids=list(range(cfg['NC'])),
                               trace=trace)
    outs = []
    for c in range(cfg['NC']):
        ngr = pp['cuts_g'][c + 1] - pp['cuts_g'][c]
        outs.append(np.asarray(res.results[c]['out']).reshape(-1)[:ngr])
    return np.concatenate(outs).astype(np.float32), res


def kernel(**inputs) -> np.ndarray:
    out, _ = run(inputs)
    return out


# revision 30
# speedup vs baseline: 1.0027x; 1.0027x over previous
"""Trainium2 Bass kernel for nn_GATRegressor (2-layer edge-featured GAT + pool + MLP).

Strategy (8-core SPMD, single NEFF):
  - Nodes partitioned into 8 graph-aligned ranges; edges partitioned by dst.
  - Per 128-dst-node tile: indirect-DMA gather of [h | al_src] rows by edge src,
    attention alpha computed on-chip (leaky-relu/exp without segment-max --
    mathematically identical softmax), scatter-add via one-hot selection-matrix
    matmul on the PE accumulating in PSUM (denominators ride along as extra
    columns). Per-head transform W applied AFTER aggregation (linearity), with
    0.25/denom folded in before the transform.
  - One AllGather of the per-core [h|al_s|al_d] shard between layers; one tiny
    AllReduce for the mean-edge-attr self-loop term.
  - Global mean-pool + 2-layer MLP on-device (counts division after W_r1 matmul).

Host side does only index/layout preprocessing (sort edges by dst, remap ids,
pad, interleave) and small weight repacking; all FLOP-heavy math runs on device.
"""
import os
import numpy as np
from contextlib import ExitStack

BODY_LEVEL = int(os.environ.get('KBODY', '9'))

import concourse.bass as bass
import concourse.bacc as bacc
import concourse.mybir as mybir
import concourse.tile as tile
from concourse.bass import IndirectOffsetOnAxis
from concourse.bass_utils import run_bass_kernel_spmd

F32 = mybir.dt.float32
I32 = mybir.dt.int32
AF = mybir.ActivationFunctionType
OP = mybir.AluOpType
P = 128
NEG = 0.2


def make_cfg(N=100000, E=400000, G=4096, NC=8, TPC=98, GPC=640, L=2, IN=64, HID=64, ED=12, H=4):
    cfg = dict(N=N, E=E, G=G, NC=NC, TPC=TPC, GPC=GPC, L=L, IN=IN, HID=HID, ED=ED, H=H)
    cfg['NPC'] = TPC * P
    cfg['NT'] = NC * cfg['NPC']
    cfg['SENT'] = cfg['NT']
    cfg['NWIN'] = GPC // P
    return cfg


FULL_CFG = make_cfg()


def _finish(nc):
    nc.finalize()
    return nc


# --------------------------------------------------------------------------
# host preprocessing: index/layout only (+ tiny weight repacking)
# --------------------------------------------------------------------------
def preprocess(inputs, cfg):
    N, E, G, NC, TPC, GPC = cfg['N'], cfg['E'], cfg['G'], cfg['NC'], cfg['TPC'], cfg['GPC']
    NPC, NT, SENT, L, H, ED = cfg['NPC'], cfg['NT'], cfg['SENT'], cfg['L'], cfg['H'], cfg['ED']
    HID = cfg['HID']

    x = np.ascontiguousarray(np.asarray(inputs['x'], np.float32))
    ei = np.asarray(inputs['edge_index'])
    ea = np.ascontiguousarray(np.asarray(inputs['edge_attr'], np.float32))
    batch = np.asarray(inputs['batch'])

    W_in = np.asarray(inputs['W_in'], np.float32)
    b_in = np.asarray(inputs['b_in'], np.float32)
    W_gat = np.asarray(inputs['W_gat'], np.float32)
    att_src = np.asarray(inputs['att_src'], np.float32)
    att_dst = np.asarray(inputs['att_dst'], np.float32)
    W_edge = np.asarray(inputs['W_edge'], np.float32)
    att_edge = np.asarray(inputs['att_edge'], np.float32)
    b_gat = np.asarray(inputs['b_gat'], np.float32)
    W_r1 = np.asarray(inputs['W_r1'], np.float32)
    b_r1 = np.asarray(inputs['b_r1'], np.float32)
    W_r2 = np.asarray(inputs['W_r2'], np.float32)
    b_r2 = np.asarray(inputs['b_r2'], np.float32)

    # ---- weight packing ----
    # W_ext_l [64, 264] = [0.25*W (256) | As_l (4) | Ad_l (4)]
    W_ext = np.zeros((L, HID, 4 * HID + 2 * H), np.float32)
    for l in range(L):
        Wl = W_gat[l].reshape(HID, H, HID)
        W_ext[l, :, :4 * HID] = 0.25 * W_gat[l]
        for h in range(H):
            W_ext[l, :, 4 * HID + h] = np.einsum('kc,c->k', Wl[:, h, :], att_src[l, h])
            W_ext[l, :, 4 * HID + H + h] = np.einsum('kc,c->k', Wl[:, h, :], att_dst[l, h])
    Ae = np.zeros((ED, L * H), np.float32)
    for l in range(L):
        Wl = W_edge[l].reshape(ED, H, HID)
        for h in range(H):
            Ae[:, l * H + h] = np.einsum('dc,c->d', Wl[:, h, :], att_edge[l, h])
    W_in_ext = np.ascontiguousarray(np.concatenate([W_in, b_in[None, :]], axis=0))

    # ---- graph-aligned node shard cuts ----
    gcounts = np.bincount(batch, minlength=G)
    cum = np.cumsum(gcounts)
    cuts_g = [0]
    for c in range(1, NC):
        cuts_g.append(int(np.searchsorted(cum, c * N / NC)))
    cuts_g.append(G)
    cuts_g = np.array(cuts_g)
    assert np.all(np.diff(cuts_g) > 0) and np.all(np.diff(cuts_g) <= GPC), np.diff(cuts_g)
    cuts_n = np.array([0] + [int(cum[cuts_g[c] - 1]) for c in range(1, NC)] + [N])
    npc_real = np.diff(cuts_n)
    assert np.all(npc_real <= NPC), npc_real

    core_of = (np.searchsorted(cuts_n, np.arange(N), side='right') - 1).astype(np.int64)
    slot_of = np.arange(N) - cuts_n[core_of]
    pad_id = (core_of * NPC + slot_of).astype(np.int32)

    # ---- edge sorting by (dst core, dst slot) ----
    src, dst = ei[0].astype(np.int64), ei[1].astype(np.int64)
    dcore = core_of[dst]
    dslot = slot_of[dst]
    order = np.lexsort((dslot, dcore))
    s_src = src[order]
    s_dslot = dslot[order]
    s_dcore = dcore[order]
    s_ea = ea[order]

    tile_of = s_dcore * TPC + (s_dslot // P)
    tcnt = np.bincount(tile_of, minlength=NC * TPC).reshape(NC, TPC)
    ct = np.maximum(np.ceil(tcnt / P).astype(np.int64).max(axis=0), 1)   # [TPC]
    CH = int(ct.sum())
    SLOT = CH * P
    ch_off = np.concatenate([[0], np.cumsum(ct)]).astype(np.int64)

    src_idx = np.full((NC, SLOT), SENT, np.int32)
    dstf = np.full((NC, SLOT), 999.0, np.float32)
    ea_slot = np.zeros((NC, SLOT, ED), np.float32)

    tile_starts = np.concatenate([[0], np.cumsum(tcnt.reshape(-1))])
    for c in range(NC):
        for t in range(TPC):
            gt = c * TPC + t
            e0, e1 = tile_starts[gt], tile_starts[gt + 1]
            n = e1 - e0
            s0 = ch_off[t] * P
            if n:
                src_idx[c, s0:s0 + n] = pad_id[s_src[e0:e1]]
                dstf[c, s0:s0 + n] = (s_dslot[e0:e1] % P).astype(np.float32)
                ea_slot[c, s0:s0 + n] = s_ea[e0:e1]

    def interleave(a):  # [NC, SLOT, ...] -> [NC, P, CH, ...] with slot s=(j*P+p) -> [p, j]
        return np.ascontiguousarray(
            a.reshape(NC, CH, P, *a.shape[2:]).transpose(0, 2, 1, *range(3, a.ndim + 1)))

    src_idx_il = interleave(src_idx)
    dstf_il = interleave(dstf)
    dstf_rows = dstf.reshape(NC, 1, SLOT)  # slot-major: [1, j*128+p]
    ea_sT = np.ascontiguousarray(ea_slot.transpose(0, 2, 1))  # [NC, 12, SLOT]

    # x transposed + ones row, per-core slices [NC, 65, NPC]
    xT = np.zeros((NC, cfg['IN'] + 1, NPC), np.float32)
    xT[:, cfg['IN']] = 1.0
    xTg = xT.reshape(NC * NPC * 0 + NC, cfg['IN'] + 1, NPC)  # alias
    for c in range(NC):
        sl = slice(cuts_n[c], cuts_n[c + 1])
        xTg[c, :cfg['IN'], :npc_real[c]] = x[sl].T

    batch_local = np.full((NC, NPC, 1), -1.0, np.float32)
    for c in range(NC):
        sl = slice(cuts_n[c], cuts_n[c + 1])
        batch_local[c, :npc_real[c], 0] = (batch[sl] - cuts_g[c]).astype(np.float32)

    wr1e = np.zeros((HID + 1, HID + 1), np.float32)
    wr1e[:HID, :HID] = W_r1
    wr1e[HID, HID] = 1.0
    return dict(W_in_ext=W_in_ext, W_ext=W_ext, Ae=Ae, b_gat=b_gat,
                W_r1=np.ascontiguousarray(wr1e), b_r1=b_r1,
                W_r2row=np.ascontiguousarray(W_r2[:, 0][None, :]), b_r2=float(b_r2.reshape(-1)[0]),
                xT=xT, src_idx=src_idx_il, dstf=dstf_il, dstf_rows=dstf_rows,
                ea_sT=ea_sT, batch_local=batch_local,
                ct=[int(v) for v in ct], ch_off=[int(v) for v in ch_off], CH=CH,
                cuts_g=cuts_g, npc_real=npc_real)


# --------------------------------------------------------------------------
# bass program
# --------------------------------------------------------------------------
def build(cfg, ct, ch_off, CH, b_r2, stage='full'):
    N, E, NC, TPC, GPC, L, H, ED = (cfg['N'], cfg['E'], cfg['NC'], cfg['TPC'],
                                    cfg['GPC'], cfg['L'], cfg['H'], cfg['ED'])
    NPC, NT, NWIN, IN, HID = cfg['NPC'], cfg['NT'], cfg['NWIN'], cfg['IN'], cfg['HID']
    HP = H * HID            # 256
    WE_C = 4 * HID + 2 * H  # 264
    RC = H * (HID + 1)      # 260 rhs/psum_B columns

    nc = bacc.Bacc(None, num_devices=NC)

    # ---- I/O ----
    t_xT = nc.dram_tensor('xt', [IN + 1, NPC], F32, kind='ExternalInput')
    t_winext = nc.dram_tensor('winext', [IN + 1, HID], F32, kind='ExternalInput')
    t_wext = nc.dram_tensor('wext', [L, HID, WE_C], F32, kind='ExternalInput')
    t_ae = nc.dram_tensor('ae', [ED, L * H], F32, kind='ExternalInput')
    t_bgat = nc.dram_tensor('bgat', [L, HID], F32, kind='ExternalInput')
    t_wr1 = nc.dram_tensor('wr1', [HID + 1, HID + 1], F32, kind='ExternalInput')
    t_br1 = nc.dram_tensor('br1', [1, HID], F32, kind='ExternalInput')
    t_wr2 = nc.dram_tensor('wr2', [1, HID], F32, kind='ExternalInput')
    t_srci = nc.dram_tensor('srci', [P, CH], I32, kind='ExternalInput')
    t_dstf = nc.dram_tensor('dstf', [P, CH], F32, kind='ExternalInput')
    t_dstfr = nc.dram_tensor('dstfr', [1, CH * P], F32, kind='ExternalInput')
    t_east = nc.dram_tensor('east', [ED, CH * P], F32, kind='ExternalInput')
    t_batch = nc.dram_tensor('batchl', [NPC, 1], F32, kind='ExternalInput')
    t_out = nc.dram_tensor('out', [GPC, 1], F32, kind='ExternalOutput')

    # ---- internal DRAM ----
    hx_sh = [nc.dram_tensor(f'hx_sh{l}', [NPC, 72], F32) for l in range(L)]
    hx_full = [nc.dram_tensor(f'hx_full{l}', [NT + 1, 72], F32)
               for l in range(L)]
    hx_last = nc.dram_tensor('hx_last', [NPC, HID + 1], F32)
    t_ale = nc.dram_tensor('ale', [P, L, CH, H], F32)
    ar_in = nc.dram_tensor('ar_in', [1, L * H], F32)
    ar_out = nc.dram_tensor('ar_out', [1, L * H], F32)

    groups = [list(range(NC))]

    with tile.TileContext(nc) as tc, ExitStack() as ctx:
        cpool = ctx.enter_context(tc.tile_pool(name='const', bufs=1))

        # ---------- constants ----------
        iotaG_const = nc.inline_tensor(
            np.broadcast_to(np.arange(GPC, dtype=np.float32), (P, GPC)).copy(), name='iotaG_c')
        iotac_const = nc.inline_tensor(np.arange(P, dtype=np.float32)[:, None].copy(),
                                       name='iotac_c')
        iotaG = cpool.tile([P, GPC], F32)
        nc.sync.dma_start(out=iotaG[:], in_=iotaG_const[:, :])
        iota128 = iotaG[:, 0:P]
        iota_col = cpool.tile([P, 1], F32)
        nc.sync.dma_start(out=iota_col[:], in_=iotac_const[:, :])
        ident = cpool.tile([P, P], F32)
        nc.vector.tensor_scalar(out=ident[:], in0=iota128, scalar1=iota_col[:, :1],
                                scalar2=None, op0=OP.is_equal)
        ones_col = cpool.tile([P, 1], F32)
        nc.vector.memset(ones_col[:], 1.0)

        w_in_sb = cpool.tile([IN + 1, HID], F32)
        nc.sync.dma_start(out=w_in_sb[:], in_=t_winext[:, :])
        wext_sb = []
        for l in range(L):
            # W_ext duplicated into partitions [0:64] and [64:128] so per-head
            # matmul lhsT slices (base partition 0 or 64) find a matching rhs.
            wl = cpool.tile([2 * HID, WE_C], F32, tag=f'wext{l}', name=f'wext_sb{l}')
            nc.sync.dma_start(out=wl[0:HID, :], in_=t_wext[l, :, :])
            nc.sync.dma_start(out=wl[HID:2 * HID, :], in_=t_wext[l, :, :])
            wext_sb.append(wl)
        ae_sb = cpool.tile([ED, L * H], F32)
        nc.sync.dma_start(out=ae_sb[:], in_=t_ae[:, :])
        wr1_sb = cpool.tile([HID + 1, HID + 1], F32)
        nc.sync.dma_start(out=wr1_sb[:], in_=t_wr1[:, :])

        ones_row = cpool.tile([1, P], F32)
        nc.vector.memset(ones_row[:], 1.0)

        def bcast_row(dst_tile, src_row_ap, width, pool):
            """dst[p, :] = src_row for all p, via K=1 matmul (ones ⊗ row)."""
            pbc = pool.tile([P, max(width, 256)], F32, tag='pT', name='pbc', space='PSUM')
            nc.tensor.matmul(out=pbc[:, 0:width], lhsT=ones_row[:], rhs=src_row_ap,
                             start=True, stop=True)
            nc.vector.tensor_copy(out=dst_tile[:], in_=pbc[:, 0:width])

        bgat_row = cpool.tile([1, L * HID], F32)
        nc.sync.dma_start(out=bgat_row[:], in_=t_bgat[:, :].rearrange('l c -> (l c)')[None, :])
        bgat_bc = cpool.tile([P, L * HID], F32)
        br1_row = cpool.tile([1, HID], F32)
        nc.sync.dma_start(out=br1_row[:], in_=t_br1[:, :])
        br1_bc = cpool.tile([P, HID], F32)
        wr2_row = cpool.tile([1, HID], F32)
        nc.sync.dma_start(out=wr2_row[:], in_=t_wr2[:, :])
        wr2_bc = cpool.tile([P, HID], F32)

        # sentinel row of hx_full: al_s=-1e9 kills padded edges (alpha=0), and the
        # h part only ever gets multiplied by that alpha=0, so -1e9 everywhere is fine.
        sent_row = cpool.tile([1, 72], F32)
        nc.vector.memset(sent_row[:], -1.0e9)
        for l in range(L):
            nc.gpsimd.dma_start(out=hx_full[l][NT:NT + 1, :], in_=sent_row[:])

        with tc.tile_pool(name='psA', bufs=2, space='PSUM') as psA, \
             tc.tile_pool(name='psB', bufs=1, space='PSUM') as psB, \
             tc.tile_pool(name='sb', bufs=6) as sb, \
             tc.tile_pool(name='gat', bufs=12) as gat:

            bcast_row(bgat_bc, bgat_row[:], L * HID, psA)
            bcast_row(br1_bc, br1_row[:], HID, psA)
            bcast_row(wr2_bc, wr2_row[:], HID, psA)

            # ---------- phase ALE: al_e for local edge slots (both layers) + local sum ----------
            ale_acc = cpool.tile([P, L * H], F32)
            nc.vector.memset(ale_acc[:], 0.0)
            GRP = 8
            for g0 in range(0, CH, GRP):
                gg = min(GRP, CH - g0)
                ea8 = sb.tile([ED, GRP * P], F32, tag='ea8')
                nc.sync.dma_start(out=ea8[:, 0:gg * P], in_=t_east[:, g0 * P:(g0 + gg) * P])
                acc8 = sb.tile([P, GRP, L * H], F32, tag='acc8')
                for jj in range(gg):
                    pale = psB.tile([P, 72], F32, tag='pO', space='PSUM')
                    nc.tensor.matmul(out=pale[:, 0:L * H], lhsT=ea8[:, jj * P:(jj + 1) * P],
                                     rhs=ae_sb[:], start=True, stop=True)
                    nc.vector.tensor_copy(out=acc8[:, jj, :], in_=pale[:, 0:L * H])
                # write both layers' slices
                for l in range(L):
                    nc.sync.dma_start(out=t_ale[:, l, g0:g0 + gg, :],
                                      in_=acc8[:, 0:gg, l * H:(l + 1) * H])
                # accumulate local sum: reduce over jj
                tmp8 = sb.tile([P, L * H], F32, tag='tmp8')
                nc.vector.tensor_reduce(out=tmp8[:], in_=acc8[:, 0:gg, :].rearrange('p j h -> p h j'),
                                        op=OP.add, axis=mybir.AxisListType.X)
                nc.vector.tensor_add(out=ale_acc[:], in0=ale_acc[:], in1=tmp8[:])
            # cross-partition sum -> [1, 8] -> AllReduce -> mean -> broadcast
            pm = psB.tile([P, 72], F32, tag='pO', space='PSUM')
            nc.tensor.matmul(out=pm[0:1, 0:L * H], lhsT=ones_col[:], rhs=ale_acc[:],
                             start=True, stop=True)
            m_sb = cpool.tile([1, L * H], F32)
            nc.vector.tensor_copy(out=m_sb[:], in_=pm[0:1, 0:L * H])
            nc.sync.dma_start(out=ar_in[:, :], in_=m_sb[:])
            tc.strict_bb_all_engine_barrier()
            nc.gpsimd.collective_compute('AllReduce', OP.add, replica_groups=groups,
                                         ins=[ar_in[:, :]], outs=[ar_out[:, :]])
            tc.strict_bb_all_engine_barrier()
            mr_sb = cpool.tile([1, L * H], F32)
            nc.sync.dma_start(out=mr_sb[:], in_=ar_out[:, :])
            nc.scalar.mul(mr_sb[:], mr_sb[:], 1.0 / E)
            alem_bc = cpool.tile([P, L * H], F32)
            bcast_row(alem_bc, mr_sb[:], L * H, psA)

            if stage == 'ale':
                nc.sync.dma_start(out=t_out[0:1, :], in_=m_sb[:, 0:1])

            # ---------- phase 0: h0 = elu(x @ W_in + b_in), al_s/al_d ----------
            for t in (range(TPC) if stage != 'ale' else ()):
                rows = slice(t * P, (t + 1) * P)
                xt = sb.tile([IN + 1, P], F32, tag='xt')
                nc.sync.dma_start(out=xt[:], in_=t_xT[:, rows])
                pb = psA.tile([P, RC], F32, tag='pB', space='PSUM')
                nc.tensor.matmul(out=pb[:, 0:HID], lhsT=xt[:], rhs=w_in_sb[:],
                                 start=True, stop=True)
                hn, hnT, alsd = _elu_transform(nc, sb, psA, psB, pb[:, 0:HID], None,
                                              ident, wext_sb[0], HID)
                nc.sync.dma_start(out=hx_sh[0][rows, 0:64], in_=hn[:])
                nc.sync.dma_start(out=hx_sh[0][rows, 64:72], in_=alsd[:])

            # AllGather layer-0 table
            if stage == 'ale':
                pass
            tc.strict_bb_all_engine_barrier()
            nc.gpsimd.collective_compute('AllGather', OP.bypass, replica_groups=groups,
                                         ins=[hx_sh[0][:, :]], outs=[hx_full[0][0:NT, :]])
            tc.strict_bb_all_engine_barrier()

            if stage == 'phase0':
                probe = sb.tile([1, 72], F32, tag='probe', name='probe')
                nc.sync.dma_start(out=probe[:], in_=hx_full[0][123:124, :])
                nc.sync.dma_start(out=t_out[0:1, :], in_=probe[:, 0:1])

            # ---------- GAT layers ----------
            ntile_cap = TPC
            lstage = stage
            if stage.startswith('layer0t'):
                ntile_cap = int(stage[len('layer0t'):])
                lstage = 'layer0'
            for l in (range(L) if lstage in ('layer0', 'full') else ()):
                if lstage == 'layer0' and l == 1:
                    probe = sb.tile([1, 72], F32, tag='probe', name='probe')
                    nc.sync.dma_start(out=probe[:], in_=hx_full[1][123:124, :])
                    nc.sync.dma_start(out=t_out[0:1, :], in_=probe[:, 0:1])
                    break
                last = (l == L - 1)
                for t in range(min(TPC, ntile_cap) if lstage == 'layer0' else TPC):
                    rows = slice(t * P, (t + 1) * P)
                    ctt = ct[t]
                    j0 = ch_off[t]
                    selft = sb.tile([P, 72], F32, tag='selft')
                    nc.sync.dma_start(out=selft[:], in_=hx_sh[l][rows, :])
                    srct = sb.tile([P, ctt], I32, tag='srct')
                    nc.sync.dma_start(out=srct[:], in_=t_srci[:, j0:j0 + ctt])
                    dft = sb.tile([P, ctt], F32, tag='dft')
                    nc.sync.dma_start(out=dft[:], in_=t_dstf[:, j0:j0 + ctt])
                    dfr = sb.tile([1, ctt * P], F32, tag='dfr')
                    nc.sync.dma_start(out=dfr[:], in_=t_dstfr[:, j0 * P:(j0 + ctt) * P])
                    alet = sb.tile([P, ctt, H], F32, tag='alet')
                    nc.sync.dma_start(out=alet[:], in_=t_ale[:, l, j0:j0 + ctt, :])

                    # gather [h|al_s|al_d] rows by edge src (one chunk per call:
                    # HW indirect DMA = one row index per partition)
                    Gt = gat.tile([P, ctt, 72], F32, tag='G')
                    for jj in range(ctt):
                        nc.gpsimd.indirect_dma_start(
                            out=Gt[:, jj, :], out_offset=None, in_=hx_full[l][:, :],
                            in_offset=IndirectOffsetOnAxis(ap=srct[:, jj:jj + 1], axis=0))

                    if BODY_LEVEL < 2:
                        continue
                    # al_d per edge: expand this tile's al_d rows through the
                    # transposed one-hot (dst_local) matrix on the PE
                    pD = psB.tile([P, 32], F32, tag='pD', space='PSUM')
                    for j0p in range(0, ctt, 2):
                        w = min(2, ctt - j0p)
                        pDst = psA.tile([P, HP], F32, tag='pT', name='pDst', space='PSUM')
                        nc.tensor.matmul(out=pDst[:, 0:w * P], lhsT=ones_row[:],
                                         rhs=dfr[:, j0p * P:(j0p + w) * P],
                                         start=True, stop=True)
                        STp = sb.tile([P, 2 * P], F32, tag='ST')
                        nc.vector.tensor_scalar(out=STp[:, 0:w * P], in0=pDst[:, 0:w * P],
                                                scalar1=iota_col[:, :1], scalar2=None,
                                                op0=OP.is_equal)
                        for q in range(w):
                            jj = j0p + q
                            nc.tensor.matmul(out=pD[:, 4 * jj:4 * jj + 4],
                                             lhsT=STp[:, q * P:(q + 1) * P],
                                             rhs=selft[:, 68:72], start=True, stop=True)

                    if BODY_LEVEL < 3:
                        continue
                    # alpha-hat per edge
                    lg = sb.tile([P, ctt, H], F32, tag='lg')
                    nc.vector.tensor_add(out=lg[:], in0=Gt[:, :, 64:68],
                                         in1=pD[:, 0:4 * ctt].rearrange('p (j h) -> p j h', h=H))
                    nc.vector.tensor_add(out=lg[:], in0=lg[:], in1=alet[:])
                    nc.vector.scalar_tensor_tensor(out=lg[:], in0=lg[:], scalar=NEG,
                                                   in1=lg[:], op0=OP.mult, op1=OP.max)
                    ah = sb.tile([P, ctt, H], F32, tag='ah')
                    nc.scalar.activation(ah[:], lg[:], AF.Exp)

                    # rhs = per-head [alpha*h (64) | alpha]
                    rhs = sb.tile([P, ctt, RC], F32, tag='rhs')
                    rhs_r = rhs[:].rearrange('p j (h c) -> p j h c', h=H)
                    for h in range(H):
                        nc.vector.tensor_mul(
                            out=rhs_r[:, :, h, 0:HID], in0=Gt[:, :, 0:HID],
                            in1=ah[:, :, h:h + 1].to_broadcast([P, ctt, HID]))
                    nc.vector.tensor_copy(out=rhs_r[:, :, :, HID], in_=ah[:])

                    if BODY_LEVEL < 4:
                        continue
                    pb = psA.tile([P, RC], F32, tag='pB', space='PSUM')
                    for jj in range(ctt):
                        S = sb.tile([P, P], F32, tag='S')
                        nc.vector.tensor_scalar(out=S[:], in0=iota128,
                                                scalar1=dft[:, jj:jj + 1], scalar2=None,
                                                op0=OP.is_equal)
                        nc.tensor.matmul(out=pb[:], lhsT=S[:], rhs=rhs[:, jj, :],
                                         start=(jj == 0), stop=(jj == ctt - 1))

                    if BODY_LEVEL < 5:
                        continue
                    # self-loop alpha
                    sl = sb.tile([P, H], F32, tag='sl')
                    nc.vector.tensor_add(out=sl[:], in0=selft[:, 64:68], in1=selft[:, 68:72])
                    nc.vector.tensor_add(out=sl[:], in0=sl[:],
                                         in1=alem_bc[:, l * H:(l + 1) * H])
                    nc.vector.scalar_tensor_tensor(out=sl[:], in0=sl[:], scalar=NEG,
                                                   in1=sl[:], op0=OP.mult, op1=OP.max)
                    ahs = sb.tile([P, H], F32, tag='ahs')
                    nc.scalar.activation(ahs[:], sl[:], AF.Exp)

                    pb_r = pb[:].rearrange('p (h c) -> p h c', h=H)
                    dn = sb.tile([P, H], F32, tag='dn')
                    nc.vector.tensor_add(out=dn[:], in0=pb_r[:, :, HID], in1=ahs[:])
                    nc.vector.tensor_scalar_add(dn[:], dn[:], 1.0e-16)
                    rc = sb.tile([P, H], F32, tag='rc')
                    nc.vector.reciprocal(rc[:], dn[:])

                    # Bw = (B + h_self * ahs) * recip   [128, 4, 64]
                    bw = sb.tile([P, H, HID], F32, tag='bw')
                    nc.vector.tensor_mul(
                        out=bw[:],
                        in0=selft[:, 0:64].rearrange('p (o c) -> p o c', o=1).to_broadcast([P, H, HID]),
                        in1=ahs[:].rearrange('p (h o) -> p h o', h=H).to_broadcast([P, H, HID]))
                    nc.vector.tensor_add(out=bw[:], in0=bw[:], in1=pb_r[:, :, 0:HID])
                    nc.vector.tensor_mul(
                        out=bw[:], in0=bw[:],
                        in1=rc[:].rearrange('p (h o) -> p h o', h=H).to_broadcast([P, H, HID]))

                    if BODY_LEVEL < 6:
                        continue
                    # transpose Bw head-by-head ([128,64] -> [64,128], full psum
                    # tiles: transpose-mode into column-sliced PSUM fails on HW),
                    # then per-head transform accumulating into one psum
                    bt = sb.tile([HID, H * P], F32, tag='bt')
                    for h in range(H):
                        pTh = psA.tile([HID, P], F32, tag='pN', name='pTh', space='PSUM')
                        nc.tensor.transpose(out=pTh[:], in_=bw[:, h, :], identity=ident[:])
                        nc.vector.tensor_copy(out=bt[:, h * P:(h + 1) * P], in_=pTh[:])
                    pO = psB.tile([P, 72], F32, tag='pO', space='PSUM')
                    for h in range(H):
                        nc.tensor.matmul(out=pO[:, 0:HID],
                                         lhsT=bt[:, h * P:(h + 1) * P],
                                         rhs=wext_sb[l][0:HID, h * HID:(h + 1) * HID],
                                         start=(h == 0), stop=(h == H - 1))

                    if BODY_LEVEL < 7:
                        continue
                    if not last:
                        hn, hnT, alsd = _elu_transform(nc, sb, psA, psB, pO[:, 0:HID],
                                                       bgat_bc[:, l * HID:(l + 1) * HID],
                                                       ident, wext_sb[l + 1], HID)
                        nc.sync.dma_start(out=hx_sh[l + 1][rows, 0:64], in_=hn[:])
                        nc.sync.dma_start(out=hx_sh[l + 1][rows, 64:72], in_=alsd[:])
                    else:
                        hn = _elu_only(nc, sb, pO[:, 0:HID], bgat_bc[:, l * HID:(l + 1) * HID])
                        hp = sb.tile([P, HID + 1], F32, tag='hp')
                        nc.vector.tensor_copy(out=hp[:, 0:HID], in_=hn[:])
                        nc.vector.memset(hp[:, HID:HID + 1], 1.0)
                        nc.sync.dma_start(out=hx_last[rows, :], in_=hp[:])

                if not last:
                    tc.strict_bb_all_engine_barrier()
                    nc.gpsimd.collective_compute(
                        'AllGather', OP.bypass, replica_groups=groups,
                        ins=[hx_sh[l + 1][:, :]], outs=[hx_full[l + 1][0:NT, :]])
                    tc.strict_bb_all_engine_barrier()

        # ---------- pooling + MLP ----------
        if stage == 'full':
            _pooling_phase(nc, tc, cfg, t_out, t_batch, hx_last, iotaG, ident,
                           wr1_sb, br1_bc, wr2_bc, b_r2)
    nc.finalize()
    return nc


def _pooling_phase(nc, tc, cfg, t_out, t_batch, hx_last, iotaG, ident,
                   wr1_sb, br1_bc, wr2_bc, b_r2):
        NC, TPC, GPC, HID = cfg['NC'], cfg['TPC'], cfg['GPC'], cfg['HID']
        NWIN = cfg['NWIN']
        with tc.tile_pool(name='psP', bufs=1, space='PSUM') as psP, \
             tc.tile_pool(name='sbp', bufs=3) as sbp:
            NWA = min(4, NWIN)  # graphs columns in first psum bank group (512)
            ppa = psP.tile([HID + 1, NWA * P], F32, tag='ppa', space='PSUM')
            if NWIN > NWA:
                ppb = psP.tile([HID + 1, (NWIN - NWA) * P], F32, tag='ppb',
                               name='ppb', space='PSUM')
            else:
                ppb = None
            for t in range(TPC):
                rows = slice(t * P, (t + 1) * P)
                lh = sbp.tile([P, HID + 1], F32, tag='lh')
                nc.sync.dma_start(out=lh[:], in_=hx_last[rows, :])
                bt_ = sbp.tile([P, 1], F32, tag='btl')
                nc.sync.dma_start(out=bt_[:], in_=t_batch[rows, :])
                Sb = sbp.tile([P, GPC], F32, tag='Sb')
                nc.vector.tensor_scalar(out=Sb[:], in0=iotaG[:], scalar1=bt_[:, :1],
                                        scalar2=None, op0=OP.is_equal)
                nc.tensor.matmul(out=ppa[:], lhsT=lh[:], rhs=Sb[:, 0:NWA * P],
                                 start=(t == 0), stop=(t == TPC - 1))
                if ppb is not None:
                    nc.tensor.matmul(out=ppb[:], lhsT=lh[:], rhs=Sb[:, NWA * P:],
                                     start=(t == 0), stop=(t == TPC - 1))
            pool_sb = sbp.tile([HID + 1, GPC], F32, tag='pool')
            nc.vector.tensor_copy(out=pool_sb[:, 0:NWA * P], in_=ppa[:])
            if ppb is not None:
                nc.vector.tensor_copy(out=pool_sb[:, NWA * P:], in_=ppb[:])

            for w in range(NWIN):
                wcols = slice(w * P, (w + 1) * P)
                # [128g, 65] = pool[0:65, wcols].T @ [W_r1 | e64]: graphs land on
                # partitions directly (no transpose op), col 64 = counts
                pt = psP.tile([P, HID + 1], F32, tag='pt', space='PSUM')
                nc.tensor.matmul(out=pt[:], lhsT=pool_sb[0:HID + 1, wcols],
                                 rhs=wr1_sb[:], start=True, stop=True)
                cnt = sbp.tile([P, 1], F32, tag='cnt')
                nc.vector.tensor_scalar_max(cnt[:], pt[:, HID:HID + 1], 1.0)
                rcc = sbp.tile([P, 1], F32, tag='rcc')
                nc.vector.reciprocal(rcc[:], cnt[:])
                rp = sbp.tile([P, HID], F32, tag='rp')
                nc.vector.tensor_scalar(out=rp[:], in0=pt[:, 0:HID], scalar1=rcc[:, :1],
                                        scalar2=None, op0=OP.mult)
                nc.vector.tensor_add(out=rp[:], in0=rp[:], in1=br1_bc[:])
                rr = sbp.tile([P, HID], F32, tag='rr')
                nc.scalar.activation(rr[:], rp[:], AF.Relu)
                scr = sbp.tile([P, HID], F32, tag='scr')
                nc.vector.tensor_mul(out=scr[:], in0=rr[:], in1=wr2_bc[:])
                ov = sbp.tile([P, 1], F32, tag='ov')
                nc.vector.tensor_reduce(out=ov[:], in_=scr[:], op=OP.add,
                                        axis=mybir.AxisListType.X)
                nc.vector.tensor_scalar_add(ov[:], ov[:], b_r2)
                nc.sync.dma_start(out=t_out[w * P:(w + 1) * P, :], in_=ov[:])


def _elu_transform(nc, sb, psA, psB, z_psum, bias_bc, ident, wext_next, HID):
    """elu(z_psum + bias) -> hn; transpose -> hnT; alsd = hnT.T @ wext_next[:,256:264]."""
    z = sb.tile([P, HID], F32, tag='z')
    if bias_bc is None:
        nc.vector.tensor_copy(out=z[:], in_=z_psum)
    else:
        nc.vector.tensor_add(out=z[:], in0=z_psum, in1=bias_bc)
    hn = _elu_only(nc, sb, z[:], None)
    pN = psA.tile([HID, P], F32, tag='pN', space='PSUM')
    nc.tensor.transpose(out=pN[:], in_=hn[:], identity=ident[:])
    hnT = sb.tile([HID, P], F32, tag='hnT')
    nc.vector.tensor_copy(out=hnT[:], in_=pN[:])
    pO = psB.tile([P, 72], F32, tag='pO', space='PSUM')
    nc.tensor.matmul(out=pO[:, 64:72], lhsT=hnT[:], rhs=wext_next[0:HID, 4 * HID:4 * HID + 8],
                     start=True, stop=True)
    alsd = sb.tile([P, 8], F32, tag='alsd')
    nc.vector.tensor_copy(out=alsd[:], in_=pO[:, 64:72])
    return hn, hnT, alsd


def _elu_only(nc, sb, z_ap, bias_bc):
    """elu(z + bias): relu(z)-1 + exp(min(z,0))."""
    HIDW = z_ap.shape[-1]
    if bias_bc is not None:
        z = sb.tile([P, HIDW], F32, tag='z')
        nc.vector.tensor_add(out=z[:], in0=z_ap, in1=bias_bc)
        z_ap = z[:]
    tm = sb.tile([P, HIDW], F32, tag='tm')
    nc.vector.tensor_scalar_min(tm[:], z_ap, 0.0)
    ex = sb.tile([P, HIDW], F32, tag='ex')
    nc.scalar.activation(ex[:], tm[:], AF.Exp)
    hn = sb.tile([P, HIDW], F32, tag='hn')
    nc.vector.tensor_scalar(out=hn[:], in0=z_ap, scalar1=0.0, scalar2=-1.0,
                            op0=OP.max, op1=OP.add)
    nc.vector.tensor_add(out=hn[:], in0=hn[:], in1=ex[:])
    return hn


# --------------------------------------------------------------------------
# entry point
# --------------------------------------------------------------------------
def _in_maps(pp, cfg):
    NC = cfg['NC']
    shared = dict(winext=pp['W_in_ext'], wext=pp['W_ext'], ae=pp['Ae'],
                  bgat=pp['b_gat'], wr1=pp['W_r1'], br1=pp['b_r1'][None, :],
                  wr2=pp['W_r2row'])
    maps = []
    for c in range(NC):
        m = dict(shared)
        m['xt'] = pp['xT'][c]
        m['srci'] = pp['src_idx'][c]
        m['dstf'] = pp['dstf'][c]
        m['dstfr'] = pp['dstf_rows'][c]
        m['east'] = pp['ea_sT'][c]
        m['batchl'] = pp['batch_local'][c]
        maps.append({k: np.ascontiguousarray(v) for k, v in m.items()})
    return maps


def run(inputs, cfg=None, trace=False):
    cfg = cfg or FULL_CFG
    pp = preprocess(inputs, cfg)
    nc = build(cfg, pp['ct'], pp['ch_off'], pp['CH'], pp['b_r2'])
    res = run_bass_kernel_spmd(nc, _in_maps(pp, cfg), core_ids=list(range(cfg['NC'])),
                               trace=trace)
    outs = []
    for c in range(cfg['NC']):
        ngr = pp['cuts_g'][c + 1] - pp['cuts_g'][c]
        outs.append(np.asarray(res.results[c]['out']).reshape(-1)[:ngr])
    return np.concatenate(outs).astype(np.float32), res


def kernel(**inputs) -> np.ndarray:
    out, _ = run(inputs)
    return out
